# revision 2
# baseline (speedup 1.0000x reference)
"""BitNet-style quantized linear layer on 8 Trainium2 NeuronCores.

Reference semantics (fp32):
    x_scale = clip(max|x| over last dim, 1e-5)          # per row of x
    x_quant = clip(round(x / x_scale * 127), -128, 127)
    w_mean  = mean(weight); w_c = weight - w_mean
    w_scale = clip(mean|w_c|, 1e-5)
    w_quant = clip(round(w_c / w_scale), -1, 1)         # ternary
    y = (x_quant @ w_quant.T) * (w_scale * x_scale / 127)

Sharding: data-parallel over rows of x (B*S = 16384 rows -> 2048 rows/core),
full weight on every core.  This minimizes HBM traffic (48 MiB/core) and
needs no collectives.  w_mean / w_scale are two global scalars precomputed
on the host (sanctioned by the problem's sharding hint); the full weight
quantization itself runs on-device on every core.

Exactness notes:
  * matmul runs in bf16: x_quant in [-127,127] and ternary w_quant are both
    exactly representable in bf16, and PSUM accumulates fp32 exactly
    (|partial sums| < 2^19), so the GEMM is bit-exact.
  * ternary quant is computed as (t > tau) + (t >= -tau) - 1 with
    tau = 0.5*w_scale (exact fp32).  For normal s, round_half_even(
    fp32_div(t, s)) clipped to [-1,1] equals sign(t)*[|t| > 0.5*s] exactly,
    so this matches the reference bit-for-bit without a divide.
  * x quant uses 127/x_scale via DVE reciprocal + the (v + 2^23) - 2^23
    round-half-even trick; divergence vs the reference's divide is ~1e-7
    relative and only perturbs |q_x| by +-1 on ~1e-5 of elements (effect on
    y is ~0.1% of one quantum -> far below tolerance).
"""

import numpy as np

R_TOTAL = 16384  # B * S
D = 2048         # D_IN == D_OUT
N_CORES = 8
R_CORE = R_TOTAL // N_CORES   # 2048 rows per core
NK = D // 128                 # 16 contraction strips
NR = R_CORE // 128            # 16 row tiles per core
NO = D // 512                 # 4 output banks of 512
MAGIC = float(1.5 * 2 ** 23)  # round-half-even offset (ulp=1 both sides)

_PROGRAM_CACHE = {}
LAST_RESULTS = None  # test harness peeks at this for profiling info


def _build_program():
    import concourse.bacc as bacc
    import concourse.mybir as mybir
    import concourse.tile as tile
    from concourse import masks
    from contextlib import ExitStack

    f32 = mybir.dt.float32
    bf16 = mybir.dt.bfloat16
    Alu = mybir.AluOpType
    Act = mybir.ActivationFunctionType

    nc = bacc.Bacc("TRN2", target_bir_lowering=False, debug=False,
                   num_devices=N_CORES)

    xs = nc.dram_tensor("xs", [R_CORE, D], f32, kind="ExternalInput")
    wt = nc.dram_tensor("wt", [D, D], f32, kind="ExternalInput")
    cst = nc.dram_tensor("cst", [128, 4], f32, kind="ExternalInput")
    ys = nc.dram_tensor("ys", [R_CORE, D], f32, kind="ExternalOutput")

    xs_ap, wt_ap, cst_ap, ys_ap = xs.ap(), wt.ap(), cst.ap(), ys.ap()

    with tile.TileContext(nc) as tc, ExitStack() as ctx:
        cpool = ctx.enter_context(tc.tile_pool(name="cpool", bufs=1))
        c_sb = cpool.tile([128, 4], f32)
        nc.sync.dma_start(c_sb[:], cst_ap[:])
        neg_mean = c_sb[:, 0:1]   # -w_mean
        tau = c_sb[:, 1:2]        # 0.5 * w_scale
        neg_tau = c_sb[:, 2:3]    # -0.5 * w_scale
        ws127 = c_sb[:, 3:4]      # w_scale / 127

        ident = cpool.tile([128, 128], bf16)
        masks.make_identity(nc, ident[:])

        # ---- weight: quantize wT strips, keep full wqT resident in SBUF ----
        wraw_pool = ctx.enter_context(tc.tile_pool(name="wraw", bufs=2))
        wtmp_pool = ctx.enter_context(tc.tile_pool(name="wtmp", bufs=2))
        wq_pool = ctx.enter_context(tc.tile_pool(name="wqp", bufs=1))
        wq = []
        for k in range(NK):
            wraw = wraw_pool.tile([128, D], f32, name="wraw")
            nc.sync.dma_start(wraw[:], wt_ap[k * 128:(k + 1) * 128, :])
            # t = w - w_mean (exact fp32)
            wcen = wtmp_pool.tile([128, D], f32, name="wcen")
            nc.scalar.activation(wcen[:], wraw[:], Act.Identity,
                                 bias=neg_mean, scale=1.0)
            # a2 = (t >= -tau) - 1  in {-1, 0}
            wa2 = wtmp_pool.tile([128, D], f32, name="wa2")
            nc.vector.tensor_scalar(wa2[:], wcen[:], neg_tau, -1.0,
                                    op0=Alu.is_ge, op1=Alu.add)
            # wq = (t > tau) + a2  in {-1, 0, 1}
            wqk = wq_pool.tile([128, D], bf16, name=f"wq{k}", tag=f"wq{k}")
            nc.vector.scalar_tensor_tensor(wqk[:], wcen[:], tau, wa2[:],
                                           op0=Alu.is_gt, op1=Alu.add)
            wq.append(wqk)

        # ---- x pipeline ----
        x_pool = ctx.enter_context(tc.tile_pool(name="xp", bufs=3))
        st_pool = ctx.enter_context(tc.tile_pool(name="stp", bufs=4))
        xq_pool = ctx.enter_context(tc.tile_pool(name="xqp", bufs=2))
        xqT_pool = ctx.enter_context(tc.tile_pool(name="xqTp", bufs=2))
        tp_psum = ctx.enter_context(
            tc.tile_pool(name="tpps", bufs=2, space="PSUM"))
        y_psum = ctx.enter_context(
            tc.tile_pool(name="yps", bufs=6, space="PSUM"))
        y_pool = ctx.enter_context(tc.tile_pool(name="yop", bufs=2))

        for r in range(NR):
            xr = x_pool.tile([128, D], f32, name="xr")
            nc.sync.dma_start(xr[:], xs_ap[r * 128:(r + 1) * 128, :])

            mx = st_pool.tile([128, 1], f32, name="mx")
            nc.vector.tensor_reduce(mx[:], xr[:], axis=mybir.AxisListType.X,
                                    op=Alu.max, apply_absolute_value=True)
            mxc = st_pool.tile([128, 1], f32, name="mxc")
            nc.vector.tensor_scalar(mxc[:], mx[:], 1e-5, None, op0=Alu.max)
            rec = st_pool.tile([128, 1], f32, name="rec")
            nc.vector.reciprocal(rec[:], mxc[:])
            comb = st_pool.tile([128, 1], f32, name="comb")
            nc.vector.tensor_scalar(comb[:], mxc[:], ws127, None,
                                    op0=Alu.mult)

            # xq = round_half_even(x * (1/s) * 127) in bf16
            xsc = x_pool.tile([128, D], f32, name="xsc")
            nc.vector.tensor_scalar(xsc[:], xr[:], rec, 127.0,
                                    op0=Alu.mult, op1=Alu.mult)
            xq = xq_pool.tile([128, D], bf16, name="xq")
            nc.vector.tensor_scalar(xq[:], xsc[:], MAGIC, MAGIC,
                                    op0=Alu.add, op1=Alu.subtract)

            # transpose xq via PE into xqT (bf16), strip by strip
            xqT = xqT_pool.tile([128, D], bf16, name="xqT")
            for h in range(2):
                pst = tp_psum.tile([128, 1024], bf16, name="pst")
                for kk in range(8):
                    k = h * 8 + kk
                    nc.tensor.transpose(pst[:, kk * 128:(kk + 1) * 128],
                                        xq[:, k * 128:(k + 1) * 128],
                                        ident[:])
                nc.scalar.copy(xqT[:, h * 1024:(h + 1) * 1024], pst[:])

            # matmul: y[r_tile, :] += sum_k xqT[k].T @ wqT[k]
            ysb = y_pool.tile([128, D], f32, name="ysb")
            for o in range(NO):
                yp = y_psum.tile([128, 512], f32, name="yp")
                for k in range(NK):
                    nc.tensor.matmul(yp[:],
                                     xqT[:, k * 128:(k + 1) * 128],
                                     wq[k][:, o * 512:(o + 1) * 512],
                                     start=(k == 0), stop=(k == NK - 1))
                # dequant rescale: y * (w_scale * x_scale / 127)
                nc.scalar.mul(ysb[:, o * 512:(o + 1) * 512], yp[:], comb)

            nc.sync.dma_start(ys_ap[r * 128:(r + 1) * 128, :], ysb[:])

    nc.compile()
    return nc


def _get_program():
    key = (R_CORE, D)
    if key not in _PROGRAM_CACHE:
        _PROGRAM_CACHE[key] = _build_program()
    return _PROGRAM_CACHE[key]


def kernel(x: np.ndarray, weight: np.ndarray, _trace: bool = False,
           **_unused) -> np.ndarray:
    global LAST_RESULTS
    from concourse import bass_utils

    x = np.asarray(x)
    weight = np.asarray(weight)
    orig_shape = x.shape
    x2d = np.ascontiguousarray(x.reshape(R_TOTAL, D).astype(np.float32,
                                                            copy=False))
    w = weight.astype(np.float32, copy=False)

    # global weight stats (host precompute, per sharding hint)
    w_mean64 = np.mean(w.astype(np.float64))
    w_mean = np.float32(w_mean64)
    w_scale = np.float32(
        max(np.mean(np.abs(w.astype(np.float64) - w_mean64)), 1e-5))
    tau = np.float32(0.5) * w_scale          # exact (power-of-2 scale)
    ws127 = np.float32(w_scale / np.float32(127.0))

    wT = np.ascontiguousarray(w.T)           # layout prep for the device
    crow = np.array([-w_mean, tau, -tau, ws127], dtype=np.float32)
    cst = np.ascontiguousarray(np.broadcast_to(crow, (128, 4)))

    nc = _get_program()
    in_maps = [
        {"xs": x2d[c * R_CORE:(c + 1) * R_CORE, :], "wt": wT, "cst": cst}
        for c in range(N_CORES)
    ]
    res = bass_utils.run_bass_kernel_spmd(
        nc, in_maps, core_ids=list(range(N_CORES)), trace=_trace)
    LAST_RESULTS = res

    y2d = np.concatenate([res.results[c]["ys"] for c in range(N_CORES)],
                         axis=0)
    return y2d.reshape(orig_shape).astype(np.float32, copy=False)
